# revision 1
# baseline (speedup 1.0000x reference)
"""Trainium2 Bass kernel for nn_Attention_31997506355363 (sparse_attention).

Sharding: 8 cores = 2 batches x 4 head-groups (4 heads of 16 each).
Each core computes its batch's full-sequence double-attend for its 4 heads,
plus the partial output projection (Wout rows for its heads); host sums the
4 head-group partials per batch.

Math notes (verified vs reference in fp64 to ~9e-7 rel):
  - mask keeps j<=i OR j>i+512  (the strip i<j<=i+512 is masked out)
  - softmax has a per-head sink logit in the denominator only
  - |sim| <= ~6.4 so softmax runs without max-subtraction: p = exp(sim),
    denom = sum_j p + exp(sink)
  - attends are computed transposed: simT[j,i] tiles -> exp -> outT
    accumulated as v.T @ p per 128-j-block (contraction always on the
    partition dim, so no attention-matrix transposes are needed, and
    attend1's output hiddensT feeds attend2 directly)
  - projection outputs bounce through DRAM; the attend working set is
    streamed back per (head, pass)
"""

import sys

for _p in ("/opt/trn_rl_repo",):
    if _p not in sys.path:
        sys.path.insert(0, _p)

import numpy as np
import concourse.bass as bass
from concourse import bacc
import concourse.mybir as mybir
from concourse.tile import TileContext
from concourse.vector_clock import ScopedClock
from concourse.masks import make_identity
import bass_rust

FP32 = mybir.dt.float32
N_CORES = 8
N = 2048            # sequence length
DQ = 1024           # model dim
HEADS = 4           # heads per core
SCALE = 0.125       # 64 ** -0.5, folded into k1T / k2T at projection copy
NB = N // 128       # 16 key blocks
PASS = 1024         # attend i-pass width (2 passes)
ACT = mybir.ActivationFunctionType

# matmul input dtype.  float32r looks 4x faster in the cost model but its
# fused 4-byte weight self-load measures ~150us per matmul on this HW
# (~250ms/body vs ~2ms with plain float32), so float32 wins decisively and
# is also bit-accurate.
MM_DT = mybir.dt.float32
DEBUG = False
REPS = 1
SKIP_GPSIMD = False  # timing experiment: drop gpsimd ops in attends (wrong results)
PROJ_ONLY = False    # timing experiment: stop after projections            # kernel-body repetitions (timing only; leave 1 for grading)


class PatchedTileContext(TileContext):
    """This walrus build rejects >1 sync-wait on the tail Drain; split the
    tail-drain waits across multiple unfusable drain instructions."""

    def _drain_and_barrier(self, tick_clock, wait_clock):
        drain_inst = self.nc.sync.drain(fusable=False)
        wait_clock.add_sem_waits(
            drain_inst.ins, ScopedClock({None: tick_clock.global_clock})
        )
        waits = list(drain_inst.ins.sync_info.on_wait or [])
        if len(waits) > 1:
            drain_inst.ins.sync_info.on_wait = waits[:1]
            for i in range(1, len(waits)):
                d2 = self.nc.sync.drain(fusable=False)
                d2.ins.sync_info = bass_rust.SyncInfo(
                    on_wait=waits[i:i + 1], on_update=[]
                )
        self.nc.all_engine_barrier()
        popped = self.nc._tile_sem_poison_stack.pop()
        assert popped is self._sem_poison
        self.nc.clear_and_free_semaphores(list(self.sems.allocated().values()))
        self.nc.all_engine_barrier()


def _bank_chunks(col, w):
    """Split [col, col+w) at 512-column PSUM bank boundaries (a matmul
    output must stay within one 2KB bank)."""
    out = []
    while w > 0:
        take = min(w, 512 - (col % 512))
        out.append((col, take))
        col += take
        w -= take
    return out


def _runs_for(jb, p):
    """i-subblock runs (in 128-col units within a 1024-wide pass) that are
    not fully masked for key-block jb.  Sub-block t covers queries
    I = 8p + t; (I, jb) is fully masked iff 1 <= jb - I <= 3."""
    skip_lo = max(0, jb - 8 * p - 3)
    skip_hi = min(8, jb - 8 * p)
    if skip_lo >= skip_hi:
        return [(0, 8)], None
    runs = []
    if skip_lo > 0:
        runs.append((0, skip_lo))
    if skip_hi < 8:
        runs.append((skip_hi, 8))
    return runs, (skip_lo, skip_hi)


def build_kernel(nc, tc, io):
    mm = nc.tensor.matmul

    def fill_fr(ap, val, width):
        # memset is not ISA-legal for float32r; affine_select with an
        # always-false predicate fills unconditionally
        nc.gpsimd.affine_select(
            out=ap, in_=ap, compare_op=mybir.AluOpType.is_ge, fill=val,
            base=-1, pattern=[[0, width]], channel_multiplier=0)

    def mmr(out, lhsT, rhs, start, stop):
        mm(out, lhsT, rhs, start=start, stop=stop)

    xq, xkv = io["xq"], io["xkv"]
    wq, wk1, wv1, wk2, wv2, wout, sink = (
        io["wq"], io["wk1"], io["wv1"], io["wk2"], io["wv2"], io["wout"],
        io["sink"],
    )
    out = io["out"]

    const = tc.alloc_tile_pool(name="const", bufs=1)
    stat = tc.alloc_tile_pool(name="stat", bufs=1)
    xin = tc.alloc_tile_pool(name="xin", bufs=1)
    xtp = tc.alloc_tile_pool(name="xt", bufs=1)
    wpool = tc.alloc_tile_pool(name="w", bufs=10)
    stg = tc.alloc_tile_pool(name="stg", bufs=3)
    kst = tc.alloc_tile_pool(name="kst", bufs=2)
    vst = tc.alloc_tile_pool(name="vst", bufs=4)
    epool = tc.alloc_tile_pool(name="e", bufs=3)
    npool = tc.alloc_tile_pool(name="nrm", bufs=2)
    osb_p = tc.alloc_tile_pool(name="osb", bufs=2)
    dram = tc.alloc_tile_pool(name="dram", bufs=1, space="DRAM")
    ps_sim = tc.alloc_tile_pool(name="ps_sim", bufs=2, space="PSUM")
    ps_av = tc.alloc_tile_pool(name="ps_av", bufs=1, space="PSUM")
    ps_ones = tc.alloc_tile_pool(name="ps_ones", bufs=1, space="PSUM")
    _pools = [const, stat, xin, xtp, wpool, stg, kst, vst, epool, npool,
              osb_p, dram, ps_sim, ps_av, ps_ones]

    # ---- constants ----
    ident = const.tile([128, 128], FP32, tag="ident", name="ident")
    make_identity(nc, ident[:])
    onescol = const.tile([128, 1], MM_DT, tag="onescol", name="onescol")
    fill_fr(onescol[:], 1.0, 1)

    sink_sb = const.tile([1, HEADS], FP32, tag="sink", name="sink")
    nc.sync.dma_start(out=sink_sb[:], in_=sink[:])
    esink = const.tile([1, HEADS], FP32, tag="esink", name="esink")
    nc.scalar.activation(esink[:], sink_sb[:], ACT.Exp)
    sinkb = const.tile([128, HEADS], FP32, tag="sinkb", name="sinkb")
    nc.gpsimd.partition_broadcast(sinkb[:], esink[0:1, :])
    ones4 = const.tile([128, HEADS], FP32, tag="ones4", name="ones4")
    nc.gpsimd.memset(ones4[:], 1.0)

    # ---- SBUF statics ----
    o2T = [stat.tile([128, N], MM_DT, tag=f"o2T{t}", name=f"o2T{t}") for t in range(2)]
    wout_sb = [stat.tile([128, DQ], MM_DT, tag=f"wo{t}", name=f"wo{t}") for t in range(2)]
    for t in range(2):
        nc.sync.dma_start(out=wout_sb[t][:], in_=wout[t * 128:(t + 1) * 128, :])

    # ---- DRAM intermediates ----
    qT_d = dram.tile([256, N], MM_DT, tag="qT_d", name="qT_d")
    k1T_d = dram.tile([256, N], MM_DT, tag="k1T_d", name="k1T_d")
    k2T_d = dram.tile([512, N], MM_DT, tag="k2T_d", name="k2T_d")
    v1_d = dram.tile([N, 512], MM_DT, tag="v1_d", name="v1_d")
    v2a_d = dram.tile([N, 65 * HEADS], MM_DT, tag="v2a_d", name="v2a_d")

    # =====================================================================
    # Phase 0+1: per 512-wide n-chunk: transpose x, run projections,
    # bounce results to DRAM.
    # =====================================================================
    def transpose_chunk(x_nat):
        """x_nat: 4 tiles [128, 1024] -> 8 kt tiles [128(dim), 512(n)]."""
        res = []
        for kt in range(8):
            ps = ps_sim.tile([128, PASS], FP32, tag="sim", name="sim")
            for nbl in range(4):
                nc.tensor.transpose(
                    ps[:, nbl * 128:(nbl + 1) * 128],
                    x_nat[nbl][:, kt * 128:(kt + 1) * 128], ident[:])
            t = xtp.tile([128, 512], MM_DT, tag=f"xt{kt}", name=f"xt{kt}")
            if kt % 2 == 0:
                nc.vector.tensor_copy(t[:], ps[:, 0:512])
            else:
                nc.scalar.copy(t[:], ps[:, 0:512])
            res.append(t)
        return res

    def load_w(w_dram, cols):
        wt = [wpool.tile([128, cols], MM_DT, tag="w", name="w") for _ in range(8)]
        for kt in range(8):
            nc.sync.dma_start(out=wt[kt][:], in_=w_dram[kt * 128:(kt + 1) * 128, :])
        return wt

    for c in range(4):                    # n-chunks of 512
        ccols = slice(c * 512, (c + 1) * 512)

        # -- xq: transpose + qT projection --
        xq_nat = []
        for nbl in range(4):
            r0 = c * 512 + nbl * 128
            t1 = xin.tile([128, DQ], FP32, tag=f"xn{nbl}", name=f"xn{nbl}")
            nc.sync.dma_start(out=t1[:], in_=xq[r0:r0 + 128, :])
            xq_nat.append(t1)
        xqT = transpose_chunk(xq_nat)

        wt = load_w(wq, 256)
        for m in range(2):
            acc = ps_sim.tile([128, PASS], FP32, tag="sim", name="sim")
            for kt in range(8):
                mmr(acc[:, 0:512], wt[kt][:, m * 128:(m + 1) * 128], xqT[kt][:],
                    start=(kt == 0), stop=(kt == 7))
            s = stg.tile([128, 512], MM_DT, tag="stg", name="stg")
            nc.vector.tensor_copy(s[:], acc[:, 0:512])
            nc.sync.dma_start(out=qT_d[m * 128:(m + 1) * 128, ccols], in_=s[:])

        # -- xkv: transpose + k1/k2/v1/v2 projections --
        xkv_nat = []
        for nbl in range(4):
            r0 = c * 512 + nbl * 128
            t2 = xin.tile([128, DQ], FP32, tag=f"xn{nbl}", name=f"xn{nbl}")
            nc.sync.dma_start(out=t2[:], in_=xkv[r0:r0 + 128, :])
            xkv_nat.append(t2)
        xkvT = transpose_chunk(xkv_nat)

        wt = load_w(wk1, 256)
        for m in range(2):
            acc = ps_sim.tile([128, PASS], FP32, tag="sim", name="sim")
            for kt in range(8):
                mmr(acc[:, 0:512], wt[kt][:, m * 128:(m + 1) * 128], xkvT[kt][:],
                    start=(kt == 0), stop=(kt == 7))
            s = stg.tile([128, 512], MM_DT, tag="stg", name="stg")
            nc.scalar.mul(s[:], acc[:, 0:512], SCALE)
            nc.sync.dma_start(out=k1T_d[m * 128:(m + 1) * 128, ccols], in_=s[:])

        wt = load_w(wk2, 512)
        for m in range(4):
            acc = ps_sim.tile([128, PASS], FP32, tag="sim", name="sim")
            for kt in range(8):
                mmr(acc[:, 0:512], wt[kt][:, m * 128:(m + 1) * 128], xkvT[kt][:],
                    start=(kt == 0), stop=(kt == 7))
            s = stg.tile([128, 512], MM_DT, tag="stg", name="stg")
            nc.scalar.mul(s[:], acc[:, 0:512], SCALE)
            nc.sync.dma_start(out=k2T_d[m * 128:(m + 1) * 128, ccols], in_=s[:])

        wt = load_w(wv1, 512)
        for nbl in range(4):
            acc = ps_sim.tile([128, PASS], FP32, tag="sim", name="sim")
            for kt in range(8):
                mmr(acc[:, 0:512], xkvT[kt][:, nbl * 128:(nbl + 1) * 128], wt[kt][:],
                    start=(kt == 0), stop=(kt == 7))
            s = stg.tile([128, 512], MM_DT, tag="stg", name="stg")
            nc.vector.tensor_copy(s[:], acc[:, 0:512])
            r0 = c * 512 + nbl * 128
            nc.sync.dma_start(out=v1_d[r0:r0 + 128, :], in_=s[:])

        wt = load_w(wv2, 256)
        for nbl in range(4):
            acc = ps_sim.tile([128, PASS], FP32, tag="sim", name="sim")
            for kt in range(8):
                mmr(acc[:, 0:256], xkvT[kt][:, nbl * 128:(nbl + 1) * 128], wt[kt][:],
                    start=(kt == 0), stop=(kt == 7))
            s = stg.tile([128, 512], MM_DT, tag="stg", name="stg")
            # pack [h*64 cols] into 65-col groups with a ones column
            sv = s[:, 0:260].rearrange("p (h c) -> p h c", h=HEADS)
            nc.vector.tensor_copy(
                sv[:, :, 0:64],
                acc[:, 0:256].rearrange("p (h c) -> p h c", h=HEADS))
            nc.vector.tensor_copy(
                sv[:, :, 64:65],
                ones4[:].rearrange("p (h c) -> p h c", h=HEADS))
            r0 = c * 512 + nbl * 128
            nc.sync.dma_start(out=v2a_d[r0:r0 + 128, :], in_=s[:, 0:260])

    if DEBUG:
        for nm, t_ in (("dbg_qT", qT_d), ("dbg_k1T", k1T_d), ("dbg_k2T", k2T_d),
                       ("dbg_v1", v1_d), ("dbg_v2a", v2a_d)):
            nc.sync.dma_start(out=io[nm].bitcast(MM_DT), in_=t_[:, :])

    def dbg_sbuf(nm, ap):
        if DEBUG and nm in io:
            nc.sync.dma_start(out=io[nm].bitcast(ap.dtype), in_=ap)

    def dbg_psum(nm, ap, rows, cols):
        if DEBUG and nm in io:
            tmp = npool.tile([rows, cols], FP32, tag="dbgt", name="dbgt")
            nc.vector.tensor_copy(tmp[:], ap)
            nc.sync.dma_start(out=io[nm], in_=tmp[:])

    if PROJ_ONLY:
        # write something to out and stop
        for nb in range(NB):
            s0 = stg.tile([128, 512], MM_DT, tag="stg", name="stg")
            nc.sync.dma_start(out=s0[:], in_=v1_d[nb * 128:(nb + 1) * 128, :])
            nc.sync.dma_start(out=out[nb * 128:(nb + 1) * 128, 0:512].bitcast(MM_DT), in_=s0[:])
        for p_ in reversed(_pools):
            p_.release()
        return

    # =====================================================================
    # Phase 2: attends (streaming q/k/v slices back from DRAM)
    # =====================================================================
    def masked_exp_av(k_h, rhs_h, v_tiles, vcols, out_ps, ones_ps, p):
        """One attend pass: for each key block jb, sim -> exp -> mask ->
        accumulate v.T @ e (and optionally the ones row)."""
        for jb in range(NB):
            simp = ps_sim.tile([128, PASS], FP32, tag="sim", name="sim")
            runs, skip = _runs_for(jb, p)
            e = epool.tile([128, PASS], MM_DT, tag="e", name="e")
            for (t0, t1) in runs:
                for (col, w) in _bank_chunks(t0 * 128, (t1 - t0) * 128):
                    mmr(simp[:, col:col + w],
                        k_h[:, jb * 128:(jb + 1) * 128],
                        rhs_h[:, col:col + w],
                        start=True, stop=True)
                nc.scalar.activation(
                    e[:, t0 * 128:t1 * 128], simp[:, t0 * 128:t1 * 128],
                    ACT.Exp)
            if skip is not None and not SKIP_GPSIMD:
                fill_fr(e[:, skip[0] * 128:skip[1] * 128], 0.0,
                        (skip[1] - skip[0]) * 128)
            td = jb - 8 * p
            if SKIP_GPSIMD:
                td = -99
            if 0 <= td < 8:   # diagonal block: keep jj <= ii
                nc.gpsimd.affine_select(
                    out=e[:, td * 128:(td + 1) * 128],
                    in_=e[:, td * 128:(td + 1) * 128],
                    compare_op=mybir.AluOpType.is_ge, fill=0.0, base=0,
                    pattern=[[1, 128]], channel_multiplier=-1)
            ta = -99 if SKIP_GPSIMD else (jb - 4 - 8 * p)
            if 0 <= ta < 8:   # jb == I+4 block: keep jj > ii
                nc.gpsimd.affine_select(
                    out=e[:, ta * 128:(ta + 1) * 128],
                    in_=e[:, ta * 128:(ta + 1) * 128],
                    compare_op=mybir.AluOpType.is_ge, fill=0.0, base=-1,
                    pattern=[[-1, 128]], channel_multiplier=1)
            for s in range(2):
                mmr(out_ps[:, s * 512:(s + 1) * 512],
                    v_tiles[jb][:, vcols.start:vcols.stop],
                    e[:, s * 512:(s + 1) * 512],
                    start=(jb == 0), stop=(jb == NB - 1))
                if ones_ps is not None:
                    mmr(ones_ps[s][:], onescol[:],
                        e[:, s * 512:(s + 1) * 512],
                        start=(jb == 0), stop=(jb == NB - 1))

    for h in range(HEADS):
        k1h = kst.tile([64, N], MM_DT, tag="k1h", name="k1h")
        nc.sync.dma_start(out=k1h[:], in_=k1T_d[64 * h:64 * h + 64, :])
        k2h = kst.tile([128, N], MM_DT, tag="k2h", name="k2h")
        nc.sync.dma_start(out=k2h[:], in_=k2T_d[128 * h:128 * h + 128, :])
        for p in range(2):
            qh = kst.tile([64, PASS], MM_DT, tag="qh", name="qh")
            nc.sync.dma_start(out=qh[:], in_=qT_d[64 * h:64 * h + 64,
                                                  p * PASS:(p + 1) * PASS])
            v1s = []
            v2s = []
            for jb in range(NB):
                t = vst.tile([128, 128], MM_DT, tag="v1s", name="v1s")
                nc.sync.dma_start(
                    out=t[:], in_=v1_d[jb * 128:(jb + 1) * 128,
                                       128 * h:128 * h + 128])
                v1s.append(t)
                t = vst.tile([128, 65], MM_DT, tag="v2s", name="v2s")
                nc.sync.dma_start(
                    out=t[:], in_=v2a_d[jb * 128:(jb + 1) * 128,
                                        65 * h:65 * h + 65])
                v2s.append(t)

            # ------------- attend 1 -------------
            out1 = ps_av.tile([128, PASS], FP32, tag="av", name="av")
            ones = [ps_ones.tile([1, 512], FP32, tag=f"ones{s_}",
                                 name=f"ones{s_}") for s_ in range(2)]
            masked_exp_av(k1h, qh, v1s, slice(0, 128), out1, ones, p)

            if h == 0 and p == 0:
                dbg_psum("dbg_out1", out1[:], 128, PASS)
                dbg_psum("dbg_ones0", ones[0][:], 1, 512)
                dbg_psum("dbg_ones1", ones[1][:], 1, 512)

            # normalize + silu -> hT
            rb = npool.tile([128, PASS], FP32, tag="rb", name="rb")
            if SKIP_GPSIMD:
                nc.vector.memset(rb[:], 1.0)
            for s_ in range(SKIP_GPSIMD and 0 or 2):
                ds_ = npool.tile([1, 512], FP32, tag=f"ds{s_}",
                                 name=f"ds{s_}")
                nc.vector.tensor_copy(ds_[:], ones[s_][:])
                nc.vector.tensor_scalar_add(ds_[:], ds_[:],
                                            sinkb[0:1, h:h + 1])
                nc.vector.reciprocal_approx_fast(ds_[:], ds_[:])
                nc.gpsimd.partition_broadcast(
                    rb[:, 512 * s_:512 * (s_ + 1)], ds_[:])
            z = npool.tile([128, PASS], FP32, tag="z", name="z")
            nc.vector.tensor_mul(z[:], out1[:], rb[:])
            tql = npool.tile([128, PASS], FP32, tag="tq", name="tq")
            nc.scalar.activation(tql[:], z[:], ACT.Exp, scale=-1.0)
            nc.vector.tensor_scalar_add(tql[:], tql[:], 1.0)
            rsb = npool.tile([128, PASS], FP32, tag="rb", name="rb")
            nc.vector.reciprocal_approx_fast(rsb[:], tql[:])
            hT = npool.tile([128, PASS], MM_DT, tag="hT", name="hT")
            nc.vector.tensor_mul(hT[:], z[:], rsb[:])
            if h == 0 and p == 0:
                dbg_sbuf("dbg_rb", rb[:])
                dbg_sbuf("dbg_z", z[:])
                dbg_sbuf("dbg_hT", hT[:])

            # ------------- attend 2 -------------
            out2 = ps_av.tile([65, PASS], FP32, tag="av", name="av")
            masked_exp_av(k2h, hT, v2s, slice(0, 65), out2, None, p)

            # normalize attend2 (denominator rode along as row 64)
            d2 = npool.tile([1, PASS], FP32, tag="dsb", name="dsb")
            nc.vector.tensor_copy(d2[:], out2[64:65, :])
            nc.vector.tensor_scalar_add(d2[:], d2[:], sinkb[0:1, h:h + 1])
            nc.vector.reciprocal_approx_fast(d2[:], d2[:])
            rb2 = npool.tile([64, PASS], FP32, tag="rb", name="rb")
            if SKIP_GPSIMD:
                nc.vector.memset(rb2[:], 1.0)
            else:
                nc.gpsimd.partition_broadcast(rb2[:], d2[0:1, :])
            dst = o2T[h // 2][64 * (h % 2):64 * (h % 2) + 64,
                             p * PASS:(p + 1) * PASS]
            nc.vector.tensor_mul(dst, out2[0:64, :], rb2[:])
            if h == 0 and p == 0:
                dbg_psum("dbg_out2", out2[:], 65, PASS)

    # =====================================================================
    # Phase 3: partial out = o2T.T @ wout
    # =====================================================================
    for nb in range(NB):
        acc = ps_av.tile([128, PASS], FP32, tag="av", name="av")
        for s in range(2):
            for kt in range(2):
                mmr(acc[:, s * 512:(s + 1) * 512],
                    o2T[kt][:, nb * 128:(nb + 1) * 128],
                    wout_sb[kt][:, s * 512:(s + 1) * 512],
                    start=(kt == 0), stop=(kt == 1))
        osb = osb_p.tile([128, DQ], FP32, tag="osb", name="osb")
        nc.vector.tensor_copy(osb[:], acc[:])
        nc.sync.dma_start(out=out[nb * 128:(nb + 1) * 128, :], in_=osb[:])

    for p_ in reversed(_pools):
        p_.release()


_NC_CACHE = {}


def build_nc():
    key = (str(MM_DT), REPS, DEBUG, SKIP_GPSIMD, PROJ_ONLY)
    if key in _NC_CACHE:
        return _NC_CACHE[key]
    nc = bacc.Bacc("TRN2", target_bir_lowering=False, debug=False,
                   num_devices=N_CORES)
    io = {
        "xq": nc.dram_tensor("xq", [N, DQ], FP32, kind="ExternalInput").ap(),
        "xkv": nc.dram_tensor("xkv", [N, DQ], FP32, kind="ExternalInput").ap(),
        "wq": nc.dram_tensor("wq", [DQ, 256], MM_DT, kind="ExternalInput").ap(),
        "wk1": nc.dram_tensor("wk1", [DQ, 256], MM_DT, kind="ExternalInput").ap(),
        "wv1": nc.dram_tensor("wv1", [DQ, 512], MM_DT, kind="ExternalInput").ap(),
        "wk2": nc.dram_tensor("wk2", [DQ, 512], MM_DT, kind="ExternalInput").ap(),
        "wv2": nc.dram_tensor("wv2", [DQ, 256], MM_DT, kind="ExternalInput").ap(),
        "wout": nc.dram_tensor("wout", [256, DQ], MM_DT, kind="ExternalInput").ap(),
        "sink": nc.dram_tensor("sink", [1, HEADS], FP32, kind="ExternalInput").ap(),
        "out": nc.dram_tensor("out", [N, DQ], FP32, kind="ExternalOutput").ap(),
    }
    if DEBUG:
        for nm, shp in (("dbg_qT", [256, N]), ("dbg_k1T", [256, N]),
                        ("dbg_k2T", [512, N]), ("dbg_v1", [N, 512]),
                        ("dbg_v2a", [N, 260]), ("dbg_out1", [128, PASS]),
                        ("dbg_ones0", [1, 512]), ("dbg_ones1", [1, 512]),
                        ("dbg_dsb0", [1, 512]), ("dbg_dsb1", [1, 512]),
                        ("dbg_rb", [128, PASS]), ("dbg_z", [128, PASS]),
                        ("dbg_hT", [128, PASS]), ("dbg_out2", [65, PASS])):
            io[nm] = nc.dram_tensor(nm, shp, FP32, kind="ExternalOutput").ap()
    with TileContext(nc) as tc:
        if REPS == 0:
            pool0 = tc.alloc_tile_pool(name="p0", bufs=1)
            t0_ = pool0.tile([128, DQ], FP32, name="t0_")
            nc.sync.dma_start(out=t0_[:], in_=io["xq"][0:128, :])
            for nb in range(NB):
                nc.sync.dma_start(out=io["out"][nb * 128:(nb + 1) * 128, :],
                                  in_=t0_[:])
            pool0.release()
        for _ in range(REPS):
            build_kernel(nc, tc, io)
    nc.compile()
    _NC_CACHE[key] = (nc, io)
    return nc, io


def make_in_maps(inputs):
    in_maps = []
    for c in range(N_CORES):
        b, g = c // 4, c % 4
        s64 = slice(g * 256, (g + 1) * 256)
        s128 = slice(g * 512, (g + 1) * 512)
        in_maps.append({
            "xq": np.ascontiguousarray(inputs["queries_input"][b]),
            "xkv": np.ascontiguousarray(inputs["key_values_input"][b]),
            "wq": np.ascontiguousarray(inputs["Wq"][:, s64]),
            "wk1": np.ascontiguousarray(inputs["Wk1"][:, s64]),
            "wv1": np.ascontiguousarray(inputs["Wv1"][:, s128]),
            "wk2": np.ascontiguousarray(inputs["Wk2"][:, s128]),
            "wv2": np.ascontiguousarray(inputs["Wv2"][:, s64]),
            "wout": np.ascontiguousarray(inputs["Wout"][s64, :]),
            "sink": np.ascontiguousarray(
                inputs["attn_sink"][g * 4:(g + 1) * 4]).reshape(1, HEADS),
        })
    return in_maps


def kernel(**inputs):
    from concourse.bass_utils import run_bass_kernel_spmd

    inputs = {k: np.asarray(v) for k, v in inputs.items()}
    nc, _ = build_nc()
    in_maps = make_in_maps(inputs)
    res = run_bass_kernel_spmd(nc, in_maps, list(range(N_CORES)))
    out = np.zeros((2, N, DQ), dtype=np.float32)
    for c in range(N_CORES):
        out[c // 4] += res.results[c]["out"]
    return out



# revision 8
# speedup vs baseline: 6164.0000x; 6164.0000x over previous
"""Loop-based fp16 Trainium2 kernel for nn_Attention_31997506355363.

Same sharding as baseline: 8 cores = 2 batches x 4 head-groups.

This execution environment charges a large per-emitted-instruction cost
(~40-80us) and a much smaller per-executed cost, so the kernel is built
from hardware For_i loops with small static bodies instead of full
unrolling (baseline: ~5k instructions -> here ~400 emitted).

Structure per core (4 heads h, all fp16 matmul operands, fp32 PSUM):
  X:  cast x to fp16 in DRAM (SWDGE cast DMA), then 8+8 DMA-transposes
      build xqT/xkvT [128, kt, 2048] in SBUF.  No PE transposes.
  P:  projections with For_i over n-chunks; weights resident in SBUF.
      qT/k1T stored head-pair-stacked [128, 2, 2048] (partitions 0-63 =
      even head, 64-127 = odd head), k2T per-head [128, 4, 2048],
      v1 natural [128, nb, 4*128], v2 natural + ones column [128, nb, 4*65].
  A:  For_i over i-chunks ci (4 x 512).  Per attend, the key-block loop is
      split into three For_i ranges so only the 8 diagonal-ish blocks pay
      masking; mask applied as multiply by a slice of a precomputed
      [128, 31*128] mask strip (block type = jb - I).  exp folds the 1/8
      qk scale and an exp(-8) bias (denominators stay in fp16 range).
      attend1 denominator: fp16 running sum of e over jb + ones-column
      matmul reduction; attend2 denominator rides as v2's 65th column.
      PSUM: S [128,2048] sim + A [128,2048] av accumulator (8 banks).
      Accumulation groups are opened/closed by zero-rhs start/stop matmuls
      so loop bodies keep static start/stop flags.
  O:  For_i over 16 row blocks: out = o2T.T @ wout -> DMA per block.
"""

import sys

for _p in ("/opt/trn_rl_repo",):
    if _p not in sys.path:
        sys.path.insert(0, _p)

import numpy as np
import concourse.bass as bass
from concourse import bacc
import concourse.mybir as mybir
from concourse.bass import ds
from concourse.tile import TileContext
from concourse.vector_clock import ScopedClock
import bass_rust

FP32 = mybir.dt.float32
FP16 = mybir.dt.float16
N_CORES = 8
N = 2048
DQ = 1024
HEADS = 4
SCALE = 0.125
EBIAS = -8.0          # exp bias: e = exp(scale*sim - 8) keeps sums in fp16
ACT = mybir.ActivationFunctionType
REPS = 1
DUMMY_COLS = 0
PHASE = 4            # 1: stop after X, 2: after projections, 3: after attends


class PatchedTileContext(TileContext):
    """This walrus build rejects >1 sync-wait on the tail Drain; split the
    tail-drain waits across multiple unfusable drain instructions."""

    def _drain_and_barrier(self, tick_clock, wait_clock):
        drain_inst = self.nc.sync.drain(fusable=False)
        wait_clock.add_sem_waits(
            drain_inst.ins, ScopedClock({None: tick_clock.global_clock})
        )
        si = drain_inst.ins.sync_info
        waits = list(si.on_wait or []) if si is not None else []
        if len(waits) > 1:
            drain_inst.ins.sync_info.on_wait = waits[:1]
            for i in range(1, len(waits)):
                d2 = self.nc.sync.drain(fusable=False)
                d2.ins.sync_info = bass_rust.SyncInfo(
                    on_wait=waits[i:i + 1], on_update=[]
                )
        self.nc.all_engine_barrier()
        popped = self.nc._tile_sem_poison_stack.pop()
        assert popped is self._sem_poison
        self.nc.clear_and_free_semaphores(list(self.sems.allocated().values()))
        self.nc.all_engine_barrier()


def build_kernel(nc, tc, io):
    mm = nc.tensor.matmul
    xq, xkv, out, sink = io["xq"], io["xkv"], io["out"], io["sink"]

    const = tc.alloc_tile_pool(name="const", bufs=1)
    stat = tc.alloc_tile_pool(name="stat", bufs=1)
    dram = tc.alloc_tile_pool(name="dram", bufs=1, space="DRAM")
    proj = tc.alloc_tile_pool(name="proj", bufs=1)      # released after P

    # ---------------- constants ----------------
    onescol = const.tile([128, 1], FP16, name="onescol")
    nc.vector.memset(onescol[:], 1.0)
    ones4 = const.tile([128, HEADS], FP16, name="ones4")
    nc.vector.memset(ones4[:], 1.0)
    ezero = const.tile([128, 512], FP16, name="ezero")
    nc.vector.memset(ezero[:], 0.0)

    ebias_b = const.tile([128, 1], FP32, name="ebias_b")
    nc.vector.memset(ebias_b[:], EBIAS)

    sink_sb = const.tile([1, HEADS], FP32, name="sink_sb")
    nc.sync.dma_start(out=sink_sb[:], in_=sink[:])
    esink4 = const.tile([1, HEADS], FP32, name="esink4")
    nc.scalar.activation(esink4[:], sink_sb[:], ACT.Exp, bias=ebias_b[0:1, :])

    # mask strip: 31 blocks of 128 cols, block u <-> d = 15-u where
    # d = jb - I.  d<0 or d>4: keep-all; d in {1,2,3}: drop-all;
    # d=0: keep j<=i; d=4: keep j>i.
    strip = const.tile([128, 31 * 128], FP16, name="strip")
    nc.vector.memset(strip[:], 1.0)
    nc.vector.memset(strip[:, 12 * 128:15 * 128], 0.0)
    # u=15 (d=0): keep col >= part  (pred: -p + c >= 0)
    nc.gpsimd.affine_select(
        out=strip[:, 15 * 128:16 * 128], in_=strip[:, 15 * 128:16 * 128],
        compare_op=mybir.AluOpType.is_ge, fill=0.0, base=0,
        pattern=[[1, 128]], channel_multiplier=-1)
    # u=11 (d=4): keep part > col  (pred: -1 + p - c >= 0)
    nc.gpsimd.affine_select(
        out=strip[:, 11 * 128:12 * 128], in_=strip[:, 11 * 128:12 * 128],
        compare_op=mybir.AluOpType.is_ge, fill=0.0, base=-1,
        pattern=[[-1, 128]], channel_multiplier=1)

    # ---------------- weights (resident) ----------------
    def wload(name, cols, pool):
        t = pool.tile([128, 8, cols], FP16, name=name)
        nc.sync.dma_start(
            out=t[:], in_=io[name].rearrange("(k p) c -> p k c", p=128))
        return t

    wq_sb = wload("wq", 256, proj)
    wk1_sb = wload("wk1", 256, proj)
    wk2_sb = wload("wk2", 512, proj)
    wv1_sb = wload("wv1", 512, proj)
    wv2_sb = wload("wv2", 256, proj)
    wout_sb = stat.tile([128, 2, DQ], FP16, name="wout_sb")
    nc.sync.dma_start(
        out=wout_sb[:], in_=io["wout"].rearrange("(k p) c -> p k c", p=128))

    # ---------------- X: cast + DMA-transpose ----------------
    xq16_d = dram.tile([N, DQ], FP16, name="xq16_d")
    xkv16_d = dram.tile([N, DQ], FP16, name="xkv16_d")
    nc.gpsimd.dma_start(out=xq16_d[:, :], in_=xq[:, :])
    nc.gpsimd.dma_start(out=xkv16_d[:, :], in_=xkv[:, :])

    xqT = proj.tile([128, 8, N], FP16, name="xqT")
    xkvT = proj.tile([128, 8, N], FP16, name="xkvT")
    for k in range(8):
        nc.sync.dma_start(out=xqT[:, k, :],
                          in_=xq16_d[:, k * 128:(k + 1) * 128], transpose=True)
        nc.sync.dma_start(out=xkvT[:, k, :],
                          in_=xkv16_d[:, k * 128:(k + 1) * 128], transpose=True)

    # ---------------- projection outputs ----------------
    qT = stat.tile([128, 2, N], FP16, name="qT")     # head-pair stacked
    k1T = stat.tile([128, 2, N], FP16, name="k1T")
    k2T = stat.tile([128, 4, N], FP16, name="k2T")   # per-head
    v1n = stat.tile([128, 16, 512], FP16, name="v1n")
    v2a = stat.tile([128, 16, 512], FP16, name="v2a")
    o2T = stat.tile([128, 2, N], FP16, name="o2T")
    hT = stat.tile([128, HEADS, 512], FP16, name="hT")

    def bail(src16):
        osb_ = stat.tile([128, 512], FP32, name="osb_")
        nc.vector.tensor_copy(osb_[:], src16)
        nc.sync.dma_start(out=out[0:128, 0:512], in_=osb_[:])

    if PHASE <= 1:
        bail(xqT[:, 0, 0:512])
        proj.release()
        dram.release()
        stat.release()
        const.release()
        return

    psP = tc.alloc_tile_pool(name="psP", bufs=1, space="PSUM")
    accT = psP.tile([128, 512], FP32, name="accT")
    accV = psP.tile([128, 512], FP32, name="accV")
    accV2 = psP.tile([128, 256], FP32, name="accV2")

    # P(a): transposed projections qT/k1T/k2T
    with tc.For_i(0, N, 512) as no:
        for w_sb, xT, dst, mtiles in (
            (wq_sb, xqT, qT, 2), (wk1_sb, xkvT, k1T, 2), (wk2_sb, xkvT, k2T, 4)
        ):
            for m in range(mtiles):
                for kt in range(8):
                    mm(accT[:], w_sb[:, kt, m * 128:(m + 1) * 128],
                       xT[:, kt, ds(no, 512)], start=(kt == 0), stop=(kt == 7))
                nc.vector.tensor_copy(dst[:, m, ds(no, 512)], accT[:])

    # P(b): natural projections v1/v2(+ones)
    xcur = proj.tile([128, 8, 128], FP16, name="xcur")
    with tc.For_i(0, 16, 1) as nbi:
        nc.vector.tensor_copy(xcur[:], xkvT[:, :, ds(nbi * 128, 128)])
        for kt in range(8):
            mm(accV[:], xcur[:, kt, :], wv1_sb[:, kt, :],
               start=(kt == 0), stop=(kt == 7))
        nc.vector.tensor_copy(v1n[:, ds(nbi, 1), :], accV[:])
        for kt in range(8):
            mm(accV2[:], xcur[:, kt, :], wv2_sb[:, kt, :],
               start=(kt == 0), stop=(kt == 7))
        dstv = v2a[:, ds(nbi, 1), 0:260].rearrange("p one (h c) -> p (one h) c", c=65)
        nc.vector.tensor_copy(
            dstv[:, :, 0:64], accV2[:].rearrange("p (h c) -> p h c", c=64))
        nc.vector.tensor_copy(
            dstv[:, :, 64:65], ones4[:].rearrange("p (h c) -> p h c", c=1))

    psP.release()
    proj.release()

    if PHASE <= 2:
        bail(v1n[:, 0, :])
        dram.release()
        stat.release()
        const.release()
        return

    # ---------------- A: attends ----------------
    work = tc.alloc_tile_pool(name="work", bufs=1)
    psA = tc.alloc_tile_pool(name="psA", bufs=1, space="PSUM")
    S = psA.tile([128, 4 * 512], FP32, name="S")     # sim, 4 banks
    A = psA.tile([128, 4 * 512], FP32, name="A")     # av acc, 4 banks

    qcur = work.tile([128, 2, 512], FP16, name="qcur")
    kcur = work.tile([128, 2, 128], FP16, name="kcur")
    kcur2 = work.tile([128, 4, 128], FP16, name="kcur2")
    vcur = work.tile([128, 512], FP16, name="vcur")
    vcur2 = work.tile([128, 260], FP16, name="vcur2")
    mcur = work.tile([128, 512], FP16, name="mcur")
    o2blk = work.tile([128, 2, 512], FP16, name="o2blk")
    wstrip = work.tile([128, 2432], FP16, name="wstrip")
    e1 = work.tile([128, 4 * 512], FP16, name="e1")
    dacc = work.tile([128, 4 * 512], FP16, name="dacc")
    densb = work.tile([1, 4 * 512], FP32, name="densb")
    rb = work.tile([128, 4 * 512], FP32, name="rb")
    rb2 = work.tile([64, 4 * 512], FP32, name="rb2")
    z = work.tile([128, 4 * 512], FP32, name="z")
    tql = work.tile([128, 4 * 512], FP32, name="tql")
    rsb = work.tile([128, 4 * 512], FP32, name="rsb")
    nc.vector.memset(vcur[:], 0.0)
    nc.vector.memset(vcur2[:], 0.0)

    def att1_body(jbo, masked):
        jbo = nc.s_assert_within(jbo, 0, 1920)
        nc.vector.tensor_copy(kcur[:], k1T[:, :, ds(jbo, 128)])
        for h in range(HEADS):
            po = (h % 2) * 64
            mm(S[:, h * 512:(h + 1) * 512],
               kcur[po:po + 64, h // 2, :],
               qcur[po:po + 64, h // 2, :], start=True, stop=True)
        nc.scalar.activation(e1[:], S[:], ACT.Exp, scale=SCALE, bias=ebias_b[:])
        if masked:
            nc.vector.tensor_copy(
                mcur[:], wstrip[:, ds(1920 - jbo, 512)])
            for h in range(HEADS):
                nc.vector.tensor_mul(
                    e1[:, h * 512:(h + 1) * 512],
                    e1[:, h * 512:(h + 1) * 512], mcur[:])
        nc.vector.tensor_copy(vcur[:], v1n[:].rearrange(
            "p nb c -> p (nb c)")[:, ds(jbo * 4, 512)])
        for h in range(HEADS):
            mm(A[:, h * 512:(h + 1) * 512], vcur[:, h * 128:(h + 1) * 128],
               e1[:, h * 512:(h + 1) * 512], start=False, stop=False)
        nc.vector.tensor_add(dacc[:], dacc[:], e1[:])

    def att2_body(jbo, masked):
        jbo = nc.s_assert_within(jbo, 0, 1920)
        nc.vector.tensor_copy(kcur2[:], k2T[:, :, ds(jbo, 128)])
        for h in range(HEADS):
            mm(S[:, h * 512:(h + 1) * 512], kcur2[:, h, :],
               hT[:, h, :], start=True, stop=True)
        nc.scalar.activation(e1[:], S[:], ACT.Exp, scale=SCALE, bias=ebias_b[:])
        if masked:
            nc.vector.tensor_copy(
                mcur[:], wstrip[:, ds(1920 - jbo, 512)])
            for h in range(HEADS):
                nc.vector.tensor_mul(
                    e1[:, h * 512:(h + 1) * 512],
                    e1[:, h * 512:(h + 1) * 512], mcur[:])
        nc.vector.tensor_copy(vcur2[:], v2a[:].rearrange(
            "p nb c -> p (nb c)")[:, ds(jbo * 4, 260)])
        for h in range(HEADS):
            mm(A[0:65, h * 512:(h + 1) * 512], vcur2[:, h * 65:(h + 1) * 65],
               e1[:, h * 512:(h + 1) * 512], start=False, stop=False)

    with tc.For_i(0, 16, 4) as ci4:
        ci4 = nc.s_assert_within(ci4, 0, 12)
        cio = ci4 * 128
        nc.vector.tensor_copy(qcur[:], qT[:, :, ds(cio, 512)])
        nc.vector.tensor_copy(wstrip[:], strip[:, ds(cio, 2432)])

        # ---- attend 1 ----
        for h in range(HEADS):   # open accumulation group with zero rhs
            mm(A[:, h * 512:(h + 1) * 512], vcur[:, h * 128:(h + 1) * 128],
               ezero[:], start=True, stop=False)
        nc.vector.memset(dacc[:], 0.0)
        with tc.For_i(0, N, 128) as jbo:
            att1_body(jbo, True)
        for h in range(HEADS):   # close group
            mm(A[:, h * 512:(h + 1) * 512], vcur[:, h * 128:(h + 1) * 128],
               ezero[:], start=False, stop=True)
        # denominator: reduce dacc over partitions, add sink, recip, bcast
        for q in range(4):
            mm(S[0:1, q * 512:(q + 1) * 512], onescol[:],
               dacc[:, q * 512:(q + 1) * 512], start=True, stop=True)
        nc.vector.tensor_copy(densb[:], S[0:1, :])
        for h in range(HEADS):
            nc.vector.tensor_scalar_add(
                densb[:, h * 512:(h + 1) * 512],
                densb[:, h * 512:(h + 1) * 512], esink4[0:1, h:h + 1])
        nc.vector.reciprocal_approx_fast(densb[:], densb[:])
        nc.gpsimd.partition_broadcast(rb[:], densb[0:1, :])
        nc.vector.tensor_mul(z[:], A[:], rb[:])
        # silu: h = z / (1 + exp(-z))
        nc.scalar.activation(tql[:], z[:], ACT.Exp, scale=-1.0)
        nc.vector.tensor_scalar_add(tql[:], tql[:], 1.0)
        nc.vector.reciprocal_approx_fast(rsb[:], tql[:])
        nc.vector.tensor_mul(
            hT[:].rearrange("p h c -> p (h c)"), z[:], rsb[:])

        # ---- attend 2 ----
        for h in range(HEADS):
            mm(A[0:65, h * 512:(h + 1) * 512],
               vcur2[:, h * 65:(h + 1) * 65], ezero[:], start=True, stop=False)
        with tc.For_i(0, N, 128) as jbo:
            att2_body(jbo, True)
        for h in range(HEADS):
            mm(A[0:65, h * 512:(h + 1) * 512],
               vcur2[:, h * 65:(h + 1) * 65], ezero[:], start=False, stop=True)
        # denominator rode along as row 64
        nc.vector.tensor_copy(densb[:], A[64:65, :])
        for h in range(HEADS):
            nc.vector.tensor_scalar_add(
                densb[:, h * 512:(h + 1) * 512],
                densb[:, h * 512:(h + 1) * 512], esink4[0:1, h:h + 1])
        nc.vector.reciprocal_approx_fast(densb[:], densb[:])
        nc.gpsimd.partition_broadcast(rb2[:], densb[0:1, :])
        for h in range(HEADS):
            po = (h % 2) * 64
            nc.vector.tensor_mul(
                o2blk[po:po + 64, h // 2, :],
                A[0:64, h * 512:(h + 1) * 512],
                rb2[0:64, h * 512:(h + 1) * 512])
        nc.vector.tensor_copy(o2T[:, :, ds(cio, 512)], o2blk[:])

    psA.release()
    work.release()

    if PHASE <= 3:
        bail(o2T[:, 0, 0:512])
        dram.release()
        stat.release()
        const.release()
        return

    # ---------------- O: output projection ----------------
    tail = tc.alloc_tile_pool(name="tail", bufs=1)
    psO = tc.alloc_tile_pool(name="psO", bufs=1, space="PSUM")
    accO = psO.tile([128, DQ], FP32, name="accO")
    ocur = tail.tile([128, 2, 128], FP16, name="ocur")
    osb = tail.tile([128, DQ], FP32, name="osb")
    with tc.For_i(0, N, 128) as nbo:
        nc.vector.tensor_copy(ocur[:], o2T[:, :, ds(nbo, 128)])
        for s in range(2):
            for kt in range(2):
                mm(accO[:, s * 512:(s + 1) * 512], ocur[:, kt, :],
                   wout_sb[:, kt, s * 512:(s + 1) * 512],
                   start=(kt == 0), stop=(kt == 1))
        nc.vector.tensor_copy(osb[:], accO[:])
        nc.sync.dma_start(out=out[ds(nbo, 128), :], in_=osb[:])
    psO.release()
    tail.release()
    dram.release()
    stat.release()
    const.release()


_NC_CACHE = {}


def build_nc():
    key = (REPS, DUMMY_COLS)
    if key in _NC_CACHE:
        return _NC_CACHE[key]
    nc = bacc.Bacc("TRN2", target_bir_lowering=False, debug=False,
                   num_devices=N_CORES)
    io = {
        "xq": nc.dram_tensor("xq", [N, DQ], FP32, kind="ExternalInput").ap(),
        "xkv": nc.dram_tensor("xkv", [N, DQ], FP32, kind="ExternalInput").ap(),
        "wq": nc.dram_tensor("wq", [DQ, 256], FP16, kind="ExternalInput").ap(),
        "wk1": nc.dram_tensor("wk1", [DQ, 256], FP16, kind="ExternalInput").ap(),
        "wv1": nc.dram_tensor("wv1", [DQ, 512], FP16, kind="ExternalInput").ap(),
        "wk2": nc.dram_tensor("wk2", [DQ, 512], FP16, kind="ExternalInput").ap(),
        "wv2": nc.dram_tensor("wv2", [DQ, 256], FP16, kind="ExternalInput").ap(),
        "wout": nc.dram_tensor("wout", [256, DQ], FP16, kind="ExternalInput").ap(),
        "sink": nc.dram_tensor("sink", [1, HEADS], FP32, kind="ExternalInput").ap(),
        "out": nc.dram_tensor("out", [N, DQ], FP32, kind="ExternalOutput").ap(),
    }
    if DUMMY_COLS:
        io["dmy"] = nc.dram_tensor(
            "dmy", [1, DUMMY_COLS], FP32, kind="ExternalInput").ap()
    with PatchedTileContext(nc) as tc:
        if DUMMY_COLS:
            pool_d = tc.alloc_tile_pool(name="pdmy", bufs=1)
            td_ = pool_d.tile([1, DUMMY_COLS], FP32, name="td_")
            nc.sync.dma_start(out=td_[:], in_=io["dmy"][:, :])
            pool_d.release()
        if REPS == 0:
            pool0 = tc.alloc_tile_pool(name="p0", bufs=1)
            t0_ = pool0.tile([128, DQ], FP32, name="t0_")
            nc.sync.dma_start(out=t0_[:], in_=io["xq"][0:128, :])
            for nb in range(16):
                nc.sync.dma_start(out=io["out"][nb * 128:(nb + 1) * 128, :],
                                  in_=t0_[:])
            pool0.release()
        for _ in range(REPS):
            build_kernel(nc, tc, io)
    nc.compile()
    _NC_CACHE[key] = (nc, io)
    return nc, io


def make_in_maps(inputs):
    in_maps = []
    for c in range(N_CORES):
        b, g = c // 4, c % 4
        s64 = slice(g * 256, (g + 1) * 256)
        s128 = slice(g * 512, (g + 1) * 512)
        in_maps.append({
            "xq": np.ascontiguousarray(inputs["queries_input"][b]),
            "xkv": np.ascontiguousarray(inputs["key_values_input"][b]),
            "wq": np.ascontiguousarray(inputs["Wq"][:, s64]).astype(np.float16),
            "wk1": np.ascontiguousarray(inputs["Wk1"][:, s64]).astype(np.float16),
            "wv1": np.ascontiguousarray(inputs["Wv1"][:, s128]).astype(np.float16),
            "wk2": np.ascontiguousarray(inputs["Wk2"][:, s128]).astype(np.float16),
            "wv2": np.ascontiguousarray(inputs["Wv2"][:, s64]).astype(np.float16),
            "wout": np.ascontiguousarray(inputs["Wout"][s64, :]).astype(np.float16),
            "sink": np.ascontiguousarray(
                inputs["attn_sink"][g * 4:(g + 1) * 4]).reshape(1, HEADS)
                .astype(np.float32),
        })
        if DUMMY_COLS:
            in_maps[-1]["dmy"] = np.zeros((1, DUMMY_COLS), np.float32)
    return in_maps


def kernel(**inputs):
    from concourse.bass_utils import run_bass_kernel_spmd

    inputs = {k: np.asarray(v) for k, v in inputs.items()}
    nc, _ = build_nc()
    in_maps = make_in_maps(inputs)
    res = run_bass_kernel_spmd(nc, in_maps, list(range(N_CORES)))
    out = np.zeros((2, N, DQ), dtype=np.float32)
    for c in range(N_CORES):
        out[c // 4] += res.results[c]["out"]
    return out
